# revision 29
# baseline (speedup 1.0000x reference)
"""Trainium2 Bass kernel for a pre-norm transformer encoder layer (SwiGLU FFN).

Shapes (hardcoded): x [2, 2048, 768], mask [2, 2048, 2048] int32,
wq/wk/wv/wo [768, 768], w1/w3 [3072, 768], w2 [768, 3072], g_attn/g_ffn [768].

Sharding: 8 cores = 2 batch x 4 query-slices of 512 tokens. Each core
computes K/V for its full batch element (replicated within the group of 4)
and attention + FFN for its own 512 tokens. No collectives.

Key techniques vs the v0 kernel:
- fp8e4 DoubleRow matmuls (contract 256/instr) for K/V/Q/w1/w3/w2. Weights
  are host-prescaled (x64 or x16) to dodge e4m3 denormals; compensations are
  folded into existing scale slots (QT scale, Silu scale, final output mul).
- Score matmuls for a head pair run concurrently in the two 64-row halves of
  the PE array (tile_position packing) into one [128,1024] PSUM tile.
- Stage 1 (projections) and stage 2 (attention) are emitted interleaved so
  the ~100us softmax-exp stream on ScalarE overlaps projection matmuls.
"""
import os
import sys

for _p in ("/opt/trn_rl_repo", "/root/.axon_site/_ro/trn_rl_repo"):
    if os.path.isdir(_p) and _p not in sys.path:
        sys.path.append(_p)

import numpy as np
import ml_dtypes

import concourse.bacc as bacc
import concourse.tile as tile
from concourse import mybir

# Prefer the combined ln+exp activation-table set so the rmsnorm/softmax
# Ln/Exp mix stays on one resident table (the default selection splits Ln
# and Exp across two sets, inserting ~23 x 1.3us ACT_TABLE_LOADs).
import concourse.hw_specs as _hw_specs

if (os.environ.get("ANT_LNEXP_PATCH", "1") == "1"
        and not getattr(_hw_specs, "_ant_lnexp_patched", False)):
    _orig_get_tables = _hw_specs.get_activation_tables

    def _get_tables_lnexp_first(arch):
        # Keep set order/IDs intact (they index act_info.json); just make
        # the combined set the ONLY one advertising Ln and Exp so the
        # table-load pass must pick it for both.
        tabs = _orig_get_tables(arch)
        combined = "natural_log_exp_and_others"
        if combined in tabs:
            out = {}
            for k, v in tabs.items():
                if k != combined:
                    v = v - {mybir.ActivationFunctionType.Ln,
                             mybir.ActivationFunctionType.Exp}
                out[k] = v
            return out
        return tabs

    _hw_specs.get_activation_tables = _get_tables_lnexp_first
    _hw_specs._ant_lnexp_patched = True
    if hasattr(bacc, "get_activation_tables"):
        bacc.get_activation_tables = _get_tables_lnexp_first

F32 = mybir.dt.float32
BF16 = mybir.dt.bfloat16
FP8 = mybir.dt.float8e4
AF = mybir.ActivationFunctionType
DR = mybir.MatmulPerfMode.DoubleRow

B, S, D, H = 2, 2048, 768, 12
DK = D // H            # 64
F = 4 * D              # 3072
T = 512                # local query tokens per core
NCH = D // 128         # 6 feature chunks
NCP = NCH // 2         # 3 feature chunk-pairs (DoubleRow)
NFC = F // 128         # 24 FFN chunks
NFP = NFC // 2         # 12 FFN chunk-pairs
NKT = S // 128         # 16 key tiles
NQT = S // T           # 4 query slices per batch element
EPS = 1e-5
WS = 64.0              # weight prescale for wq/wk/wv/w1/w2
WS3 = 16.0             # weight prescale for w3 (keeps prod under fp8 max)
WSV = 16.0             # weight prescale for wv (V values must stay < 240)


def build_nc():
    nc = bacc.Bacc("TRN2", target_bir_lowering=False, debug=False, num_devices=8)

    # host-side layouts (see prep_inputs):
    xc = nc.dram_tensor("xc", [NCP, 128, 2, S], BF16, kind="ExternalInput").ap()
    xlocT = nc.dram_tensor("xlocT", [NCH, 128, T], F32, kind="ExternalInput").ap()
    maskd = nc.dram_tensor("maskd", [NKT, 128, T], BF16,
                           kind="ExternalInput").ap()
    wqc = nc.dram_tensor("wqc", [128, NCP, 2, D], FP8, kind="ExternalInput").ap()
    wkc = nc.dram_tensor("wkc", [128, NCP, 2, D], FP8, kind="ExternalInput").ap()
    wvc = nc.dram_tensor("wvc", [128, NCP, 2, D], FP8, kind="ExternalInput").ap()
    woT = nc.dram_tensor("woT", [H, DK, D], FP8, kind="ExternalInput").ap()
    w1c = nc.dram_tensor("w1c", [NFC, 128, NCP, 256], FP8,
                         kind="ExternalInput").ap()
    w3c = nc.dram_tensor("w3c", [NFC, 128, NCP, 256], FP8,
                         kind="ExternalInput").ap()
    w2c = nc.dram_tensor("w2c", [NCH, 128, NFP, 256], FP8,
                         kind="ExternalInput").ap()
    ones16 = nc.dram_tensor("ones16", [128, 128], BF16, kind="ExternalInput").ap()

    outT = nc.dram_tensor("outT", [NCH, 128, T], F32, kind="ExternalOutput").ap()

    sim_silu = os.environ.get("BASS_SIM_SILU") == "1"
    debug_taps = os.environ.get("BASS_DEBUG_TAPS") == "1"
    taps = {}
    if debug_taps:
        for nm, shape, dt in [
            ("tap_qt", [128, T], BF16), ("tap_kt", [128, T], BF16),
            ("tap_xn", [128, 2 * T], FP8), ("tap_pr", [128, 2 * T], BF16),
            ("tap_va", [128, H * (DK + 1)], FP8), ("tap_at", [DK, T], BF16),
            ("tap_h", [128, T], F32), ("tap_hn", [128, 2 * T], FP8),
            ("tap_prod", [128, 2 * T], FP8),
        ]:
            taps[nm] = nc.dram_tensor(nm, shape, dt, kind="ExternalOutput").ap()

    with tile.TileContext(nc) as tc:
        with tc.tile_pool(name="glob", bufs=1) as Pg:
            ones16_t = Pg.tile([128, 128], BF16, name="ones16_t")
            nc.sync.dma_start(ones16_t[:], ones16)
            eps_t = Pg.tile([128, 1], F32, name="eps_t")
            nc.vector.memset(eps_t[:], EPS)
            xloc = [Pg.tile([128, T], F32, name=f"xloc{c}") for c in range(NCH)]
            hT = [Pg.tile([128, T], F32, name=f"hT{c}") for c in range(NCH)]

            # ---------------- merged stage 1+2 ---------------------------
            with (
                tc.tile_pool(name="s12", bufs=1) as P1,
                tc.tile_pool(name="ps12", bufs=1, space="PSUM") as PS1,
            ):
                # x slices first: they head the critical path (rmsnorm -> Q/K)
                xcts = {}

                def xct_dma(s):
                    ssl = slice(s * T, (s + 1) * T)
                    for cp in range(NCP):
                        xt = P1.tile([128, 2 * T], BF16, tag="xct", bufs=6,
                                     name=f"xct{cp}_{s}")
                        nc.sync.dma_start(
                            xt[:].rearrange("p (two n) -> p two n", two=2),
                            xc[cp][:, :, ssl])
                        xcts[(s, cp)] = xt

                wq_t = P1.tile([128, NCP, 2, D], FP8, name="wq_t")
                wk_t = P1.tile([128, NCP, 2, D], FP8, name="wk_t")
                wv_t = P1.tile([128, NCP, 2, D], FP8, name="wv_t")
                xct_dma(0)
                xct_dma(1)
                nc.sync.dma_start(wq_t[:], wqc)
                nc.sync.dma_start(wk_t[:], wkc)
                xct_dma(2)
                xct_dma(3)
                nc.sync.dma_start(wv_t[:], wvc)
                maskd_t = [P1.tile([128, T], BF16, name=f"maskd{kt}")
                           for kt in range(NKT)]
                for kt in range(NKT):
                    nc.sync.dma_start(maskd_t[kt][:], maskd[kt])
                for c in range(NCH):
                    nc.sync.dma_start(xloc[c][:], xlocT[c])
                wo_t = [P1.tile([DK, D], FP8, name=f"wo{h}") for h in range(H)]
                for h in range(H):
                    nc.sync.dma_start(wo_t[h][:], woT[h])
                # FFN weight streams (glob pool so their DMAs are not gated
                # on the s12->s4 pool barrier)
                w1s = [Pg.tile([128, NCP, 256], FP8, tag="w1s", bufs=6,
                               name=f"w1s{f}") for f in range(NFC)]
                w3s = [Pg.tile([128, NCP, 256], FP8, tag="w3s", bufs=6,
                               name=f"w3s{f}") for f in range(NFC)]
                w2s = [Pg.tile([128, NFP, 256], FP8, tag="w2s", bufs=2,
                               name=f"w2s{do}") for do in range(NCH)]
                for f in range(NFC):
                    nc.sync.dma_start(w1s[f][:], w1c[f])
                    nc.sync.dma_start(w3s[f][:], w3c[f])
                for do in range(NCH):
                    nc.sync.dma_start(w2s[do][:], w2c[do])
                KT = [P1.tile([128, S], BF16, name=f"KT{c}") for c in range(NCH)]
                QT = [P1.tile([128, T], BF16, name=f"QT{c}") for c in range(NCH)]
                VA = [P1.tile([128, H * (DK + 1)], FP8, name=f"VA{t}")
                      for t in range(NKT)]
                xnc = [P1.tile([128, 2 * T], FP8, name=f"xnc{cp}_{s}")
                       for s in range(NQT) for cp in range(NCP)]
                attnT = [P1.tile([DK, T], BF16, name=f"attnT{h}")
                         for h in range(H)]

                # ---- rmsnorm per slice (interleaved with Q/K0 below) ----
                def rmsnorm(s):
                    xct = [xcts[(s, cp)] for cp in range(NCP)]
                    ps_ms = PS1.tile([128, T], F32, tag="mqkv", bufs=2,
                                     name=f"ps_ms{s}")
                    for cp in range(NCP):
                        sq = P1.tile([128, 2 * T], BF16, tag="sq", bufs=2,
                                     name=f"sq{cp}_{s}")
                        nc.vector.tensor_mul(sq[:], xct[cp][:], xct[cp][:])
                        nc.tensor.matmul(ps_ms[:], ones16_t[:], sq[:, 0:T],
                                         start=(cp == 0), stop=False)
                        nc.tensor.matmul(ps_ms[:], ones16_t[:], sq[:, T:2 * T],
                                         start=False, stop=(cp == NCP - 1))
                    lntmp = P1.tile([128, T], F32, tag="lntmp", bufs=2,
                                    name=f"ln{s}")
                    nc.scalar.activation(lntmp[:], ps_ms[:], AF.Ln,
                                         bias=eps_t[:], scale=1.0 / D)
                    rstd = P1.tile([128, T], BF16, tag="rstd", bufs=2,
                                   name=f"rstd{s}")
                    nc.scalar.activation(rstd[:], lntmp[:], AF.Exp, scale=-0.5)
                    for cp in range(NCP):
                        xn = xnc[s * NCP + cp]
                        nc.vector.tensor_mul(xn[:, 0:T], xct[cp][:, 0:T],
                                             rstd[:])
                        nc.vector.tensor_mul(xn[:, T:2 * T], xct[cp][:, T:2 * T],
                                             rstd[:])

                def xn_dr(s, cp):
                    return xnc[s * NCP + cp][:].rearrange(
                        "p (two n) -> p two n", two=2)

                def warm(n, name):
                    """Keep-alive matmuls so the PE HAM stays at full clock."""
                    wp = PS1.tile([128, T], F32, tag="mqkv", bufs=2,
                                  name=f"warm_{name}")
                    for i in range(n):
                        nc.tensor.matmul(wp[:, 0:128], ones16_t[:], ones16_t[:],
                                         start=(i == 0), stop=(i == n - 1))

                def q_proj():
                    # Q for the local slice (= rotated slice 0)
                    for do in range(NCH):
                        ps_q = PS1.tile([128, T], F32, tag="mqkv", bufs=2,
                                        name=f"ps_q{do}")
                        dsl = slice(do * 128, (do + 1) * 128)
                        for cp in range(NCP):
                            nc.tensor.matmul(ps_q[:], wq_t[:, cp, :, dsl],
                                             xn_dr(0, cp), perf_mode=DR,
                                             start=(cp == 0),
                                             stop=(cp == NCP - 1))
                        # scores = (64K)^T (64Q) / 8 -> QT carries 1/(8*64*64)
                        nc.vector.tensor_scalar_mul(QT[do][:], ps_q[:],
                                                    1.0 / 32768.0)

                def k_proj(do, s):
                    """K for head-pair chunk `do`, key-slice s."""
                    ps_k = PS1.tile([128, T], F32, tag="mqkv", bufs=2,
                                    name=f"ps_k{do}_{s}")
                    dsl = slice(do * 128, (do + 1) * 128)
                    for cp in range(NCP):
                        nc.tensor.matmul(ps_k[:], wk_t[:, cp, :, dsl],
                                         xn_dr(s, cp), perf_mode=DR,
                                         start=(cp == 0), stop=(cp == NCP - 1))
                    nc.vector.tensor_copy(KT[do][:, s * T:(s + 1) * T], ps_k[:])

                def v_proj(s):
                    """V (token-major) for key-slice s -> VA tiles."""
                    for tt in range(4):
                        gt = s * 4 + tt
                        tsl = slice(tt * 128, (tt + 1) * 128)
                        ps_va = PS1.tile([128, T], F32, tag="mqkv", bufs=2,
                                         name=f"ps_va{gt}")
                        ps_vb = PS1.tile([128, T], F32, tag="mqkv", bufs=2,
                                         name=f"ps_vb{gt}")
                        for cp in range(NCP):
                            nc.tensor.matmul(
                                ps_va[:],
                                xn_dr(s, cp)[:, :, tsl],
                                wv_t[:, cp, :, 0:512], perf_mode=DR,
                                start=(cp == 0), stop=(cp == NCP - 1))
                            nc.tensor.matmul(
                                ps_vb[:, 0:256],
                                xn_dr(s, cp)[:, :, tsl],
                                wv_t[:, cp, :, 512:768], perf_mode=DR,
                                start=(cp == 0), stop=(cp == NCP - 1))
                        va = VA[gt][:].rearrange("p (h e) -> p h e", e=DK + 1)
                        nc.vector.memset(va[:, :, DK:DK + 1], 1.0)
                        nc.scalar.copy(
                            va[:, 0:8, 0:DK],
                            ps_va[:].rearrange("p (h d) -> p h d", d=DK))
                        nc.scalar.copy(
                            va[:, 8:12, 0:DK],
                            ps_vb[:, 0:256].rearrange("p (h d) -> p h d", d=DK))

                probs = {}

                def sc_block(pc, kts):
                    """Packed score pair-matmuls + exp + mask for key tiles."""
                    r0, r1 = slice(0, DK), slice(DK, 128)
                    for kt in kts:
                        ps_ab = PS1.tile([128, 2 * T], F32, tag="psab", bufs=2,
                                         name=f"ps_ab{pc}_{kt}")
                        ksl = slice(kt * 128, (kt + 1) * 128)
                        nc.tensor.matmul(ps_ab[:, 0:T], KT[pc][r0, ksl],
                                         QT[pc][r0, :], start=True, stop=True,
                                         tile_position=(0, 0))
                        nc.tensor.matmul(ps_ab[:, T:2 * T], KT[pc][r1, ksl],
                                         QT[pc][r1, :], start=True, stop=True,
                                         tile_position=(64, 0))
                        pr = P1.tile([128, 2 * T], BF16, tag="probs", bufs=12,
                                     name=f"probs{pc}_{kt}")
                        nc.scalar.activation(pr[:], ps_ab[:], AF.Exp)
                        prv = pr[:].rearrange("p (two n) -> p two n", two=2)
                        mb = maskd_t[kt][:].rearrange(
                            "p (one n) -> p one n", one=1).broadcast_to(
                            [128, 2, T])
                        nc.vector.tensor_mul(prv, prv, mb)
                        probs[(pc, kt)] = pr

                accs = {}

                def av_block(pc, kts, start, stop):
                    for par in range(2):
                        h = 2 * pc + par
                        if start:
                            accs[h] = PS1.tile([DK + 1, T], F32,
                                               tag=f"acc{par}", bufs=1,
                                               name=f"acc{h}")
                        hsl = slice(h * (DK + 1), (h + 1) * (DK + 1))
                        n = len(kts)
                        for i, kt in enumerate(kts):
                            nc.tensor.matmul(
                                accs[h][:], VA[kt][:, hsl],
                                probs[(pc, kt)][:, par * T:(par + 1) * T],
                                start=(start and i == 0),
                                stop=(stop and i == n - 1))

                def norm_block(pc):
                    """softmax denominators + attnT for the head pair."""
                    for par in range(2):
                        h = 2 * pc + par
                        lnrow = P1.tile([1, T], F32, tag="lnrow", bufs=1,
                                        name=f"lnrow{h}")
                        # acc row DK holds den; values are WSV x
                        nc.scalar.activation(lnrow[:], accs[h][DK:DK + 1, :],
                                             AF.Ln, scale=WSV)
                        srow = P1.tile([1, T], F32, tag="srow", bufs=1,
                                       name=f"srow{h}")
                        nc.scalar.activation(srow[:], lnrow[:], AF.Exp,
                                             scale=-1.0)
                        bc_sb = P1.tile([DK, T], F32, tag="bc_sb", bufs=2,
                                        name=f"bc_sb{h}")
                        nc.gpsimd.partition_broadcast(bc_sb[:], srow[:])
                        nc.vector.tensor_mul(attnT[h][:], accs[h][0:DK, :],
                                             bc_sb[:])

                def wo_partial(pc):
                    """Fold this head pair into the hT accumulators."""
                    for do in range(NCH):
                        ps_h2 = PS1.tile([128, T], F32, tag="mqkv", bufs=2,
                                         name=f"ps_h2_{pc}_{do}")
                        dsl = slice(do * 128, (do + 1) * 128)
                        for par in range(2):
                            h = 2 * pc + par
                            nc.tensor.matmul(ps_h2[:], wo_t[h][:, dsl],
                                             attnT[h][:], start=(par == 0),
                                             stop=(par == 1))
                        if pc == 0:
                            nc.vector.tensor_copy(hT[do][:], ps_h2[:])
                        else:
                            nc.vector.tensor_add(hT[do][:], ps_h2[:],
                                                 hT[do][:])

                # ---- interleaved emission: K/scores/V/AV staircase ----
                warm(40, "init")
                rmsnorm(0)
                warm(8, "w_r0")
                q_proj()
                warm(8, "w_q")
                k_proj(0, 0)
                rmsnorm(1); k_proj(0, 1)
                warm(8, "w_r1")
                rmsnorm(2); k_proj(0, 2)
                warm(8, "w_r2")
                rmsnorm(3); k_proj(0, 3)
                warm(8, "w_r3")
                sc_block(0, range(0, 16))
                v_proj(0); v_proj(1)
                av_block(0, range(0, 8), start=True, stop=False)
                v_proj(2); v_proj(3)
                for s in range(NQT):
                    k_proj(1, s)
                av_block(0, range(8, 16), start=False, stop=True)
                norm_block(0)
                wo_partial(0)
                sc_block(1, range(0, 16))
                for pc in range(2, NCH):
                    for s in range(NQT):
                        k_proj(pc, s)
                    av_block(pc - 1, range(0, 16), start=True, stop=True)
                    norm_block(pc - 1)
                    wo_partial(pc - 1)
                    warm(12, f"pc{pc}")
                    sc_block(pc, range(0, 16))
                av_block(5, range(0, 16), start=True, stop=True)
                norm_block(5)
                wo_partial(5)
                warm(24, "prewo")

                if debug_taps:
                    nc.sync.dma_start(taps["tap_qt"], QT[0][:])
                    nc.sync.dma_start(taps["tap_kt"], KT[0][:, 0:T])
                    nc.sync.dma_start(taps["tap_xn"], xnc[0][:])
                    nc.sync.dma_start(taps["tap_pr"], probs[(0, 0)][:])
                    nc.sync.dma_start(taps["tap_va"], VA[0][:])
                    nc.sync.dma_start(taps["tap_at"], attnT[0][:])

                # ---- finalize residual: hT = hT/64 + xloc ----
                for do in range(NCH):
                    h2tmp = P1.tile([128, T], F32, tag="h2tmp", bufs=2,
                                    name=f"h2tmp{do}")
                    nc.scalar.mul(h2tmp[:], hT[do][:], 1.0 / WS)
                    nc.vector.tensor_add(hT[do][:], h2tmp[:], xloc[do][:])

            # ---------------- stage 3+4: FFN ------------------------------
            with (
                tc.tile_pool(name="s4", bufs=1) as P4,
                tc.tile_pool(name="ps4", bufs=1, space="PSUM") as PS4,
            ):
                wp4 = PS4.tile([128, T], F32, tag="ps_y", bufs=2,
                               name="warm_s4")
                for i in range(24):
                    nc.tensor.matmul(wp4[:, 0:128], ones16_t[:], ones16_t[:],
                                     start=(i == 0), stop=(i == 23))
                ps_ms2 = PS4.tile([128, T], F32, tag="ps_y", bufs=2,
                                  name="ps_ms2")
                for do in range(NCH):
                    sqh = P4.tile([128, T], BF16, tag="sqh", bufs=2,
                                  name=f"sqh{do}")
                    nc.scalar.activation(sqh[:], hT[do][:], AF.Square)
                    nc.tensor.matmul(ps_ms2[:], ones16_t[:], sqh[:],
                                     start=(do == 0), stop=(do == NCH - 1))
                lntmp2 = P4.tile([128, T], F32, name="lntmp2")
                nc.scalar.activation(lntmp2[:], ps_ms2[:], AF.Ln,
                                     bias=eps_t[:], scale=1.0 / D)
                rstd2 = P4.tile([128, T], F32, name="rstd2")
                nc.scalar.activation(rstd2[:], lntmp2[:], AF.Exp, scale=-0.5)
                hnc = [P4.tile([128, 2 * T], FP8, name=f"hnc{cp}")
                       for cp in range(NCP)]
                for cp in range(NCP):
                    nc.vector.tensor_mul(hnc[cp][:, 0:T], hT[2 * cp][:],
                                         rstd2[:])
                    nc.vector.tensor_mul(hnc[cp][:, T:2 * T], hT[2 * cp + 1][:],
                                         rstd2[:])

                def hn_dr(cp):
                    return hnc[cp][:].rearrange("p (two n) -> p two n", two=2)

                if debug_taps:
                    nc.sync.dma_start(taps["tap_h"], hT[0][:])
                    nc.sync.dma_start(taps["tap_hn"], hnc[0][:])

                prodc = [P4.tile([128, 2 * T], FP8, name=f"prodc{fp}")
                         for fp in range(NFP)]
                for f in range(NFC):
                    w1_t = w1s[f]
                    w3_t = w3s[f]
                    ps_u = PS4.tile([128, T], F32, tag="ps_u", bufs=2,
                                    name=f"ps_u{f}")
                    ps_w = PS4.tile([128, T], F32, tag="ps_w", bufs=2,
                                    name=f"ps_w{f}")
                    for cp in range(NCP):
                        w1v = w1_t[:, cp, :].rearrange("p (two n) -> p two n",
                                                       two=2)
                        w3v = w3_t[:, cp, :].rearrange("p (two n) -> p two n",
                                                       two=2)
                        nc.tensor.matmul(ps_u[:], w1v, hn_dr(cp), perf_mode=DR,
                                         start=(cp == 0), stop=(cp == NCP - 1))
                        nc.tensor.matmul(ps_w[:], w3v, hn_dr(cp), perf_mode=DR,
                                         start=(cp == 0), stop=(cp == NCP - 1))
                    silu = P4.tile([128, T], BF16, tag="silu", bufs=2,
                                   name=f"silu{f}")
                    if sim_silu:
                        # CoreSim has no Silu; emulate as u*sigmoid(u)
                        nc.scalar.activation(silu[:], ps_u[:], AF.Sigmoid,
                                             scale=1.0 / WS)
                        smul = P4.tile([128, T], BF16, tag="smul", bufs=2,
                                       name=f"smul{f}")
                        nc.vector.tensor_scalar_mul(smul[:], ps_u[:], 1.0 / WS)
                        nc.vector.tensor_mul(silu[:], silu[:], smul[:])
                    else:
                        nc.scalar.activation(silu[:], ps_u[:], AF.Silu,
                                             scale=1.0 / WS)
                    # prod = silu(u) * (16 w3 hn) -> 16x true, fp8
                    nc.vector.tensor_mul(prodc[f // 2][:, (f % 2) * T:
                                                       (f % 2 + 1) * T],
                                         silu[:], ps_w[:])

                if debug_taps:
                    nc.sync.dma_start(taps["tap_prod"], prodc[0][:])

                for do in range(NCH):
                    w2_t = w2s[do]
                    ps_y = PS4.tile([128, T], F32, tag="ps_y", bufs=2,
                                    name=f"ps_y{do}")
                    for fp in range(NFP):
                        w2v = w2_t[:, fp, :].rearrange("p (two n) -> p two n",
                                                       two=2)
                        nc.tensor.matmul(ps_y[:], w2v,
                                         prodc[fp][:].rearrange(
                                             "p (two n) -> p two n", two=2),
                                         perf_mode=DR,
                                         start=(fp == 0), stop=(fp == NFP - 1))
                    # ps_y = 64*16*ffn -> scale back then residual
                    ytmp = P4.tile([128, T], F32, tag="ytmp", bufs=2,
                                   name=f"ytmp{do}")
                    nc.scalar.mul(ytmp[:], ps_y[:], 1.0 / (WS * WS3))
                    outt = P4.tile([128, T], F32, tag="outt", bufs=2,
                                   name=f"outt{do}")
                    nc.vector.tensor_add(outt[:], ytmp[:], hT[do][:])
                    nc.sync.dma_start(outT[do], outt[:])

    nc.compile()
    return nc


def prep_inputs(x, mask, wq, wk, wv, wo, w1, w2, w3, g_attn, g_ffn):
    """Build the 8 per-core input maps (host-side sharding + layout)."""
    bf = ml_dtypes.bfloat16
    f8 = ml_dtypes.float8_e4m3

    def q8(a, scale):
        return np.clip(a * scale, -240.0, 240.0).astype(f8)

    # wXc[p, cp, i, n] = ws * g[d] * wX[n, d],  d = (2cp+i)*128 + p
    def qkv_pack(w, ws):
        wt = (w * g_attn[None, :]).T.reshape(NCP, 2, 128, D)  # [cp, i, p, n]
        return np.ascontiguousarray(q8(wt.transpose(2, 0, 1, 3), ws))

    wqc = qkv_pack(wq, WS)
    wkc = qkv_pack(wk, WS)
    wvc = qkv_pack(wv, WSV)
    woTe = np.ascontiguousarray(q8(wo.T.reshape(H, DK, D), WS))

    # w1c[f, p, cp, (i, m)] = 64 * g_ffn[d] * w1[f*128+m, d]
    def ffn_pack(w, ws):
        wt = (w * g_ffn[None, :]).T.reshape(NCP, 2, 128, NFC, 128)
        return np.ascontiguousarray(
            q8(wt.transpose(3, 2, 0, 1, 4), ws).reshape(NFC, 128, NCP, 256))

    w1ce = ffn_pack(w1, WS)
    w3ce = ffn_pack(w3, WS3)
    # w2c[do, p, fp, (i, n)] = 64 * w2[do*128+n, (2fp+i)*128 + p]
    w2t = w2.T.reshape(NFP, 2, 128, NCH, 128)           # [fp, i, p, do, n]
    w2ce = np.ascontiguousarray(
        q8(w2t.transpose(3, 2, 0, 1, 4), WS).reshape(NCH, 128, NFP, 256))
    ones16 = np.ones((128, 128), bf)

    in_maps = []
    for core in range(8):
        b, qt = core // NQT, core % NQT
        # rotate tokens so the local 512-query slice is always quarter 0
        order = (np.arange(S) + qt * T) % S
        xb = x[b][order]                       # [S, D] rotated
        # xc[cp, p, i, t] = xb[t, (2cp+i)*128 + p]
        xce = np.ascontiguousarray(
            xb.T.reshape(NCP, 2, 128, S).transpose(0, 2, 1, 3)).astype(bf)
        xlocTe = np.ascontiguousarray(
            xb[0:T].T.reshape(NCH, 128, T)).astype(np.float32)
        # maskd[kt, p, i*T+q] = mask[b, qt*T+q, k], k = kt*128+p rotated keys
        msl = mask[b, qt * T:(qt + 1) * T][:, order]     # [T(q), S(k)] rotated
        maskde = np.ascontiguousarray(
            msl.T.reshape(NKT, 128, T)).astype(bf)       # [kt, p, q]
        in_maps.append({
            "xc": xce, "xlocT": xlocTe, "maskd": maskde,
            "wqc": wqc, "wkc": wkc, "wvc": wvc, "woT": woTe,
            "w1c": w1ce, "w3c": w3ce, "w2c": w2ce,
            "ones16": ones16,
        })
    return in_maps


_NC_CACHE = None


def get_nc():
    global _NC_CACHE
    if _NC_CACHE is None:
        _NC_CACHE = build_nc()
    return _NC_CACHE


def gather_output(results):
    out = np.empty((B, S, D), np.float32)
    for core in range(8):
        b, qt = core // NQT, core % NQT
        o = results[core]["outT"]              # [NCH, 128, T]
        out[b, qt * T:(qt + 1) * T, :] = o.reshape(D, T).T
    return out


def kernel(**inputs):
    from concourse.bass_utils import run_bass_kernel_spmd
    in_maps = prep_inputs(
        np.asarray(inputs["x"]), np.asarray(inputs["mask"]),
        np.asarray(inputs["wq"]), np.asarray(inputs["wk"]),
        np.asarray(inputs["wv"]), np.asarray(inputs["wo"]),
        np.asarray(inputs["w1"]), np.asarray(inputs["w2"]),
        np.asarray(inputs["w3"]),
        np.asarray(inputs["g_attn"]), np.asarray(inputs["g_ffn"]))
    nc = get_nc()
    res = run_bass_kernel_spmd(nc, in_maps, core_ids=list(range(8)))
    return gather_output(res.results)


# revision 30
# speedup vs baseline: 1.1403x; 1.1403x over previous
"""Trainium2 Bass kernel for a pre-norm transformer encoder layer (SwiGLU FFN).

Shapes (hardcoded): x [2, 2048, 768], mask [2, 2048, 2048] int32,
wq/wk/wv/wo [768, 768], w1/w3 [3072, 768], w2 [768, 3072], g_attn/g_ffn [768].

Sharding: 8 cores = 2 batch x 4 query-slices of 512 tokens. Each core
computes K/V for its full batch element (replicated within the group of 4)
and attention + FFN for its own 512 tokens. No collectives.

Key techniques vs the v0 kernel:
- fp8e4 DoubleRow matmuls (contract 256/instr) for K/V/Q/w1/w3/w2. Weights
  are host-prescaled (x64 or x16) to dodge e4m3 denormals; compensations are
  folded into existing scale slots (QT scale, Silu scale, final output mul).
- Score matmuls for a head pair run concurrently in the two 64-row halves of
  the PE array (tile_position packing) into one [128,1024] PSUM tile.
- Stage 1 (projections) and stage 2 (attention) are emitted interleaved so
  the ~100us softmax-exp stream on ScalarE overlaps projection matmuls.
"""
import os
import sys

for _p in ("/opt/trn_rl_repo", "/root/.axon_site/_ro/trn_rl_repo"):
    if os.path.isdir(_p) and _p not in sys.path:
        sys.path.append(_p)

import numpy as np
import ml_dtypes

import concourse.bacc as bacc
import concourse.tile as tile
from concourse import mybir

# Prefer the combined ln+exp activation-table set so the rmsnorm/softmax
# Ln/Exp mix stays on one resident table (the default selection splits Ln
# and Exp across two sets, inserting ~23 x 1.3us ACT_TABLE_LOADs).
import concourse.hw_specs as _hw_specs

if (os.environ.get("ANT_LNEXP_PATCH", "1") == "1"
        and not getattr(_hw_specs, "_ant_lnexp_patched", False)):
    _orig_get_tables = _hw_specs.get_activation_tables

    def _get_tables_lnexp_first(arch):
        # Keep set order/IDs intact (they index act_info.json); just make
        # the combined set the ONLY one advertising Ln and Exp so the
        # table-load pass must pick it for both.
        tabs = _orig_get_tables(arch)
        combined = "natural_log_exp_and_others"
        if combined in tabs:
            out = {}
            for k, v in tabs.items():
                if k != combined:
                    v = v - {mybir.ActivationFunctionType.Ln,
                             mybir.ActivationFunctionType.Exp}
                out[k] = v
            return out
        return tabs

    _hw_specs.get_activation_tables = _get_tables_lnexp_first
    _hw_specs._ant_lnexp_patched = True
    if hasattr(bacc, "get_activation_tables"):
        bacc.get_activation_tables = _get_tables_lnexp_first

F32 = mybir.dt.float32
BF16 = mybir.dt.bfloat16
FP8 = mybir.dt.float8e4
AF = mybir.ActivationFunctionType
DR = mybir.MatmulPerfMode.DoubleRow

B, S, D, H = 2, 2048, 768, 12
DK = D // H            # 64
F = 4 * D              # 3072
T = 512                # local query tokens per core
NCH = D // 128         # 6 feature chunks
NCP = NCH // 2         # 3 feature chunk-pairs (DoubleRow)
NFC = F // 128         # 24 FFN chunks
NFP = NFC // 2         # 12 FFN chunk-pairs
NKT = S // 128         # 16 key tiles
NQT = S // T           # 4 query slices per batch element
EPS = 1e-5
WS = 64.0              # weight prescale for wq/wk/wv/w1/w2
WS3 = 16.0             # weight prescale for w3 (keeps prod under fp8 max)
WSV = 16.0             # weight prescale for wv (V values must stay < 240)


def build_nc():
    nc = bacc.Bacc("TRN2", target_bir_lowering=False, debug=False, num_devices=8)

    # host-side layouts (see prep_inputs):
    xc = nc.dram_tensor("xc", [NCP, 128, 2, S], BF16, kind="ExternalInput").ap()
    xlocT = nc.dram_tensor("xlocT", [NCH, 128, T], F32, kind="ExternalInput").ap()
    maskd = nc.dram_tensor("maskd", [NKT, 128, T], BF16,
                           kind="ExternalInput").ap()
    wqc = nc.dram_tensor("wqc", [128, NCP, 2, D], FP8, kind="ExternalInput").ap()
    wkc = nc.dram_tensor("wkc", [128, NCP, 2, D], FP8, kind="ExternalInput").ap()
    wvc = nc.dram_tensor("wvc", [128, NCP, 2, D], FP8, kind="ExternalInput").ap()
    woT = nc.dram_tensor("woT", [NCH, 128, D], FP8, kind="ExternalInput").ap()
    w1c = nc.dram_tensor("w1c", [NFC, 128, NCP, 256], FP8,
                         kind="ExternalInput").ap()
    w3c = nc.dram_tensor("w3c", [NFC, 128, NCP, 256], FP8,
                         kind="ExternalInput").ap()
    w2c = nc.dram_tensor("w2c", [NCH, 128, NFP, 256], FP8,
                         kind="ExternalInput").ap()
    ones16 = nc.dram_tensor("ones16", [128, 128], BF16, kind="ExternalInput").ap()

    outT = nc.dram_tensor("outT", [NCH, 128, T], F32, kind="ExternalOutput").ap()

    sim_silu = os.environ.get("BASS_SIM_SILU") == "1"
    debug_taps = os.environ.get("BASS_DEBUG_TAPS") == "1"
    taps = {}
    if debug_taps:
        for nm, shape, dt in [
            ("tap_qt", [128, T], BF16), ("tap_kt", [128, T], BF16),
            ("tap_xn", [128, 2 * T], FP8), ("tap_pr", [128, 2 * T], BF16),
            ("tap_va", [128, H * (DK + 1)], FP8), ("tap_at", [DK, T], BF16),
            ("tap_h", [128, T], F32), ("tap_hn", [128, 2 * T], FP8),
            ("tap_prod", [128, 2 * T], FP8),
        ]:
            taps[nm] = nc.dram_tensor(nm, shape, dt, kind="ExternalOutput").ap()

    with tile.TileContext(nc) as tc:
        with tc.tile_pool(name="glob", bufs=1) as Pg:
            ones16_t = Pg.tile([128, 128], BF16, name="ones16_t")
            nc.sync.dma_start(ones16_t[:], ones16)
            eps_t = Pg.tile([128, 1], F32, name="eps_t")
            nc.vector.memset(eps_t[:], EPS)
            xloc = [Pg.tile([128, T], F32, name=f"xloc{c}") for c in range(NCH)]
            hT = [Pg.tile([128, T], F32, name=f"hT{c}") for c in range(NCH)]

            # ---------------- merged stage 1+2 ---------------------------
            with (
                tc.tile_pool(name="s12", bufs=1) as P1,
                tc.tile_pool(name="ps12", bufs=1, space="PSUM") as PS1,
            ):
                # x slices first: they head the critical path (rmsnorm -> Q/K)
                xcts = {}

                def xct_dma(s):
                    ssl = slice(s * T, (s + 1) * T)
                    for cp in range(NCP):
                        xt = P1.tile([128, 2 * T], BF16, tag="xct", bufs=6,
                                     name=f"xct{cp}_{s}")
                        nc.sync.dma_start(
                            xt[:].rearrange("p (two n) -> p two n", two=2),
                            xc[cp][:, :, ssl])
                        xcts[(s, cp)] = xt

                wq_t = P1.tile([128, NCP, 2, D], FP8, name="wq_t")
                wk_t = P1.tile([128, NCP, 2, D], FP8, name="wk_t")
                wv_t = P1.tile([128, NCP, 2, D], FP8, name="wv_t")
                xct_dma(0)
                xct_dma(1)
                nc.sync.dma_start(wq_t[:], wqc)
                nc.sync.dma_start(wk_t[:], wkc)
                xct_dma(2)
                xct_dma(3)
                nc.sync.dma_start(wv_t[:], wvc)
                maskd_t = [P1.tile([128, T], BF16, name=f"maskd{kt}")
                           for kt in range(NKT)]
                for kt in range(NKT):
                    nc.sync.dma_start(maskd_t[kt][:], maskd[kt])
                for c in range(NCH):
                    nc.sync.dma_start(xloc[c][:], xlocT[c])
                wo_t = [P1.tile([128, D], FP8, name=f"wo{pc}")
                        for pc in range(NCH)]
                for pc in range(NCH):
                    nc.sync.dma_start(wo_t[pc][:], woT[pc])
                # FFN weight streams (glob pool so their DMAs are not gated
                # on the s12->s4 pool barrier)
                w1s = [Pg.tile([128, NCP, 256], FP8, tag="w1s", bufs=6,
                               name=f"w1s{f}") for f in range(NFC)]
                w3s = [Pg.tile([128, NCP, 256], FP8, tag="w3s", bufs=6,
                               name=f"w3s{f}") for f in range(NFC)]
                w2s = [Pg.tile([128, NFP, 256], FP8, tag="w2s", bufs=2,
                               name=f"w2s{do}") for do in range(NCH)]
                for f in range(NFC):
                    nc.sync.dma_start(w1s[f][:], w1c[f])
                    nc.sync.dma_start(w3s[f][:], w3c[f])
                for do in range(NCH):
                    nc.sync.dma_start(w2s[do][:], w2c[do])
                KT = [P1.tile([128, S], BF16, name=f"KT{c}") for c in range(NCH)]
                QT = [P1.tile([128, T], BF16, name=f"QT{c}") for c in range(NCH)]
                VA = [P1.tile([128, H * (DK + 1)], FP8, name=f"VA{t}")
                      for t in range(NKT)]
                xnc = [P1.tile([128, 2 * T], FP8, name=f"xnc{cp}_{s}")
                       for s in range(NQT) for cp in range(NCP)]
                attnP = [P1.tile([128, T], BF16, name=f"attnP{pc}")
                         for pc in range(NCH)]

                # ---- rmsnorm per slice (interleaved with Q/K0 below) ----
                def rmsnorm(s):
                    xct = [xcts[(s, cp)] for cp in range(NCP)]
                    ps_ms = PS1.tile([128, T], F32, tag="mqkv", bufs=2,
                                     name=f"ps_ms{s}")
                    for cp in range(NCP):
                        sq = P1.tile([128, 2 * T], BF16, tag="sq", bufs=2,
                                     name=f"sq{cp}_{s}")
                        nc.vector.tensor_mul(sq[:], xct[cp][:], xct[cp][:])
                        nc.tensor.matmul(ps_ms[:], ones16_t[:], sq[:, 0:T],
                                         start=(cp == 0), stop=False)
                        nc.tensor.matmul(ps_ms[:], ones16_t[:], sq[:, T:2 * T],
                                         start=False, stop=(cp == NCP - 1))
                    lntmp = P1.tile([128, T], F32, tag="lntmp", bufs=2,
                                    name=f"ln{s}")
                    nc.scalar.activation(lntmp[:], ps_ms[:], AF.Ln,
                                         bias=eps_t[:], scale=1.0 / D)
                    rstd = P1.tile([128, T], BF16, tag="rstd", bufs=2,
                                   name=f"rstd{s}")
                    nc.scalar.activation(rstd[:], lntmp[:], AF.Exp, scale=-0.5)
                    for cp in range(NCP):
                        xn = xnc[s * NCP + cp]
                        nc.vector.tensor_mul(xn[:, 0:T], xct[cp][:, 0:T],
                                             rstd[:])
                        nc.vector.tensor_mul(xn[:, T:2 * T], xct[cp][:, T:2 * T],
                                             rstd[:])

                def xn_dr(s, cp):
                    return xnc[s * NCP + cp][:].rearrange(
                        "p (two n) -> p two n", two=2)

                def warm(n, name):
                    """Keep-alive matmuls so the PE HAM stays at full clock."""
                    wp = PS1.tile([128, T], F32, tag="mqkv", bufs=2,
                                  name=f"warm_{name}")
                    for i in range(n):
                        nc.tensor.matmul(wp[:, 0:128], ones16_t[:], ones16_t[:],
                                         start=(i == 0), stop=(i == n - 1))

                def q_proj():
                    # Q for the local slice (= rotated slice 0)
                    for do in range(NCH):
                        ps_q = PS1.tile([128, T], F32, tag="mqkv", bufs=2,
                                        name=f"ps_q{do}")
                        dsl = slice(do * 128, (do + 1) * 128)
                        for cp in range(NCP):
                            nc.tensor.matmul(ps_q[:], wq_t[:, cp, :, dsl],
                                             xn_dr(0, cp), perf_mode=DR,
                                             start=(cp == 0),
                                             stop=(cp == NCP - 1))
                        # scores = (64K)^T (64Q) / 8 -> QT carries 1/(8*64*64)
                        nc.vector.tensor_scalar_mul(QT[do][:], ps_q[:],
                                                    1.0 / 32768.0)

                def k_proj(do, s):
                    """K for head-pair chunk `do`, key-slice s."""
                    ps_k = PS1.tile([128, T], F32, tag="mqkv", bufs=2,
                                    name=f"ps_k{do}_{s}")
                    dsl = slice(do * 128, (do + 1) * 128)
                    for cp in range(NCP):
                        nc.tensor.matmul(ps_k[:], wk_t[:, cp, :, dsl],
                                         xn_dr(s, cp), perf_mode=DR,
                                         start=(cp == 0), stop=(cp == NCP - 1))
                    nc.vector.tensor_copy(KT[do][:, s * T:(s + 1) * T], ps_k[:])

                def v_proj(s):
                    """V (token-major) for key-slice s -> VA tiles."""
                    for tt in range(4):
                        gt = s * 4 + tt
                        tsl = slice(tt * 128, (tt + 1) * 128)
                        ps_va = PS1.tile([128, T], F32, tag="mqkv", bufs=2,
                                         name=f"ps_va{gt}")
                        ps_vb = PS1.tile([128, T], F32, tag="mqkv", bufs=2,
                                         name=f"ps_vb{gt}")
                        for cp in range(NCP):
                            nc.tensor.matmul(
                                ps_va[:],
                                xn_dr(s, cp)[:, :, tsl],
                                wv_t[:, cp, :, 0:512], perf_mode=DR,
                                start=(cp == 0), stop=(cp == NCP - 1))
                            nc.tensor.matmul(
                                ps_vb[:, 0:256],
                                xn_dr(s, cp)[:, :, tsl],
                                wv_t[:, cp, :, 512:768], perf_mode=DR,
                                start=(cp == 0), stop=(cp == NCP - 1))
                        va = VA[gt][:].rearrange("p (h e) -> p h e", e=DK + 1)
                        nc.vector.memset(va[:, :, DK:DK + 1], 1.0)
                        nc.scalar.copy(
                            va[:, 0:8, 0:DK],
                            ps_va[:].rearrange("p (h d) -> p h d", d=DK))
                        nc.vector.tensor_copy(
                            va[:, 8:12, 0:DK],
                            ps_vb[:, 0:256].rearrange("p (h d) -> p h d", d=DK))

                probs = {}

                def sc_block(pc, kts):
                    """Packed score pair-matmuls + exp + mask for key tiles."""
                    r0, r1 = slice(0, DK), slice(DK, 128)
                    for kt in kts:
                        ps_ab = PS1.tile([128, 2 * T], F32, tag="psab", bufs=2,
                                         name=f"ps_ab{pc}_{kt}")
                        ksl = slice(kt * 128, (kt + 1) * 128)
                        nc.tensor.matmul(ps_ab[:, 0:T], KT[pc][r0, ksl],
                                         QT[pc][r0, :], start=True, stop=True,
                                         tile_position=(0, 0))
                        nc.tensor.matmul(ps_ab[:, T:2 * T], KT[pc][r1, ksl],
                                         QT[pc][r1, :], start=True, stop=True,
                                         tile_position=(64, 0))
                        pr = P1.tile([128, 2 * T], BF16, tag="probs", bufs=12,
                                     name=f"probs{pc}_{kt}")
                        nc.scalar.activation(pr[:], ps_ab[:], AF.Exp)
                        prv = pr[:].rearrange("p (two n) -> p two n", two=2)
                        mb = maskd_t[kt][:].rearrange(
                            "p (one n) -> p one n", one=1).broadcast_to(
                            [128, 2, T])
                        nc.vector.tensor_mul(prv, prv, mb)
                        probs[(pc, kt)] = pr

                accs = {}

                def av_block(pc, kts, start, stop):
                    for par in range(2):
                        h = 2 * pc + par
                        if start:
                            accs[h] = PS1.tile([DK + 1, T], F32,
                                               tag=f"acc{par}", bufs=1,
                                               name=f"acc{h}")
                        hsl = slice(h * (DK + 1), (h + 1) * (DK + 1))
                        n = len(kts)
                        for i, kt in enumerate(kts):
                            nc.tensor.matmul(
                                accs[h][:], VA[kt][:, hsl],
                                probs[(pc, kt)][:, par * T:(par + 1) * T],
                                start=(start and i == 0),
                                stop=(stop and i == n - 1))

                def norm_block(pc):
                    """softmax denominators + attnT for the head pair.

                    Even head lands in attnP[pc][0:64]; the odd head is
                    written at partitions 0-63 then shifted to 64-127 by a
                    ScalarE copy so wo can contract the pair in one go."""
                    for par in range(2):
                        h = 2 * pc + par
                        lnrow = P1.tile([1, T], F32, tag="lnrow", bufs=1,
                                        name=f"lnrow{h}")
                        # acc row DK holds den; values are WSV x
                        nc.scalar.activation(lnrow[:], accs[h][DK:DK + 1, :],
                                             AF.Ln, scale=WSV)
                        srow = P1.tile([1, T], F32, tag="srow", bufs=1,
                                       name=f"srow{h}")
                        nc.scalar.activation(srow[:], lnrow[:], AF.Exp,
                                             scale=-1.0)
                        bc_sb = P1.tile([DK, T], F32, tag="bc_sb", bufs=2,
                                        name=f"bc_sb{h}")
                        nc.gpsimd.partition_broadcast(bc_sb[:], srow[:])
                        if par == 0:
                            nc.vector.tensor_mul(attnP[pc][0:DK, :],
                                                 accs[h][0:DK, :], bc_sb[:])
                        else:
                            odd = P1.tile([DK, T], BF16, tag="oddT", bufs=2,
                                          name=f"oddT{h}")
                            nc.vector.tensor_mul(odd[:], accs[h][0:DK, :],
                                                 bc_sb[:])
                            nc.scalar.copy(attnP[pc][DK:128, :], odd[:])

                # ---- interleaved emission: K/scores/V/AV staircase ----
                warm(40, "init")
                rmsnorm(0)
                warm(8, "w_r0")
                q_proj()
                warm(8, "w_q")
                k_proj(0, 0)
                rmsnorm(1); k_proj(0, 1)
                warm(8, "w_r1")
                rmsnorm(2); k_proj(0, 2)
                warm(8, "w_r2")
                rmsnorm(3); k_proj(0, 3)
                warm(8, "w_r3")
                sc_block(0, range(0, 16))
                v_proj(0); v_proj(1)
                av_block(0, range(0, 8), start=True, stop=False)
                k_proj(1, 0); k_proj(1, 1)
                v_proj(2)
                k_proj(1, 2); k_proj(1, 3)
                v_proj(3)
                av_block(0, range(8, 16), start=False, stop=True)
                norm_block(0)
                sc_block(1, range(0, 16))
                for pc in range(2, NCH):
                    for s in range(NQT):
                        k_proj(pc, s)
                    av_block(pc - 1, range(0, 16), start=True, stop=True)
                    norm_block(pc - 1)
                    warm(8, f"pc{pc}")
                    sc_block(pc, range(0, 16))
                av_block(5, range(0, 16), start=True, stop=True)
                norm_block(5)
                warm(48, "prewo")

                if debug_taps:
                    nc.sync.dma_start(taps["tap_qt"], QT[0][:])
                    nc.sync.dma_start(taps["tap_kt"], KT[0][:, 0:T])
                    nc.sync.dma_start(taps["tap_xn"], xnc[0][:])
                    nc.sync.dma_start(taps["tap_pr"], probs[(0, 0)][:])
                    nc.sync.dma_start(taps["tap_va"], VA[0][:])
                    nc.sync.dma_start(taps["tap_at"], attnP[0][0:DK, :])

                # ---- wo projection (pair-contract 128) + residual ----
                for do in range(NCH):
                    ps_h2 = PS1.tile([128, T], F32, tag="mqkv", bufs=2,
                                     name=f"ps_h2_{do}")
                    dsl = slice(do * 128, (do + 1) * 128)
                    for pc in range(NCH):
                        nc.tensor.matmul(ps_h2[:], wo_t[pc][:, dsl],
                                         attnP[pc][:], start=(pc == 0),
                                         stop=(pc == NCH - 1))
                    h2tmp = P1.tile([128, T], F32, tag="h2tmp", bufs=2,
                                    name=f"h2tmp{do}")
                    nc.scalar.mul(h2tmp[:], ps_h2[:], 1.0 / WS)
                    nc.vector.tensor_add(hT[do][:], h2tmp[:], xloc[do][:])

            # ---------------- stage 3+4: FFN ------------------------------
            with (
                tc.tile_pool(name="s4", bufs=1) as P4,
                tc.tile_pool(name="ps4", bufs=1, space="PSUM") as PS4,
            ):
                wp4 = PS4.tile([128, T], F32, tag="ps_y", bufs=2,
                               name="warm_s4")
                for i in range(24):
                    nc.tensor.matmul(wp4[:, 0:128], ones16_t[:], ones16_t[:],
                                     start=(i == 0), stop=(i == 23))
                ps_ms2 = PS4.tile([128, T], F32, tag="ps_y", bufs=2,
                                  name="ps_ms2")
                for do in range(NCH):
                    sqh = P4.tile([128, T], BF16, tag="sqh", bufs=2,
                                  name=f"sqh{do}")
                    nc.scalar.activation(sqh[:], hT[do][:], AF.Square)
                    nc.tensor.matmul(ps_ms2[:], ones16_t[:], sqh[:],
                                     start=(do == 0), stop=(do == NCH - 1))
                lntmp2 = P4.tile([128, T], F32, name="lntmp2")
                nc.scalar.activation(lntmp2[:], ps_ms2[:], AF.Ln,
                                     bias=eps_t[:], scale=1.0 / D)
                rstd2 = P4.tile([128, T], F32, name="rstd2")
                nc.scalar.activation(rstd2[:], lntmp2[:], AF.Exp, scale=-0.5)
                hnc = [P4.tile([128, 2 * T], FP8, name=f"hnc{cp}")
                       for cp in range(NCP)]
                for cp in range(NCP):
                    nc.vector.tensor_mul(hnc[cp][:, 0:T], hT[2 * cp][:],
                                         rstd2[:])
                    nc.vector.tensor_mul(hnc[cp][:, T:2 * T], hT[2 * cp + 1][:],
                                         rstd2[:])

                def hn_dr(cp):
                    return hnc[cp][:].rearrange("p (two n) -> p two n", two=2)

                if debug_taps:
                    nc.sync.dma_start(taps["tap_h"], hT[0][:])
                    nc.sync.dma_start(taps["tap_hn"], hnc[0][:])

                prodc = [P4.tile([128, 2 * T], FP8, name=f"prodc{fp}")
                         for fp in range(NFP)]
                for f in range(NFC):
                    w1_t = w1s[f]
                    w3_t = w3s[f]
                    ps_u = PS4.tile([128, T], F32, tag="ps_u", bufs=2,
                                    name=f"ps_u{f}")
                    ps_w = PS4.tile([128, T], F32, tag="ps_w", bufs=2,
                                    name=f"ps_w{f}")
                    for cp in range(NCP):
                        w1v = w1_t[:, cp, :].rearrange("p (two n) -> p two n",
                                                       two=2)
                        w3v = w3_t[:, cp, :].rearrange("p (two n) -> p two n",
                                                       two=2)
                        nc.tensor.matmul(ps_u[:], w1v, hn_dr(cp), perf_mode=DR,
                                         start=(cp == 0), stop=(cp == NCP - 1))
                        nc.tensor.matmul(ps_w[:], w3v, hn_dr(cp), perf_mode=DR,
                                         start=(cp == 0), stop=(cp == NCP - 1))
                    silu = P4.tile([128, T], BF16, tag="silu", bufs=2,
                                   name=f"silu{f}")
                    if sim_silu:
                        # CoreSim has no Silu; emulate as u*sigmoid(u)
                        nc.scalar.activation(silu[:], ps_u[:], AF.Sigmoid,
                                             scale=1.0 / WS)
                        smul = P4.tile([128, T], BF16, tag="smul", bufs=2,
                                       name=f"smul{f}")
                        nc.vector.tensor_scalar_mul(smul[:], ps_u[:], 1.0 / WS)
                        nc.vector.tensor_mul(silu[:], silu[:], smul[:])
                    else:
                        nc.scalar.activation(silu[:], ps_u[:], AF.Silu,
                                             scale=1.0 / WS)
                    # prod = silu(u) * (16 w3 hn) -> 16x true, fp8
                    nc.vector.tensor_mul(prodc[f // 2][:, (f % 2) * T:
                                                       (f % 2 + 1) * T],
                                         silu[:], ps_w[:])

                if debug_taps:
                    nc.sync.dma_start(taps["tap_prod"], prodc[0][:])

                for do in range(NCH):
                    w2_t = w2s[do]
                    ps_y = PS4.tile([128, T], F32, tag="ps_y", bufs=2,
                                    name=f"ps_y{do}")
                    for fp in range(NFP):
                        w2v = w2_t[:, fp, :].rearrange("p (two n) -> p two n",
                                                       two=2)
                        nc.tensor.matmul(ps_y[:], w2v,
                                         prodc[fp][:].rearrange(
                                             "p (two n) -> p two n", two=2),
                                         perf_mode=DR,
                                         start=(fp == 0), stop=(fp == NFP - 1))
                    # ps_y = 64*16*ffn -> scale back then residual
                    ytmp = P4.tile([128, T], F32, tag="ytmp", bufs=2,
                                   name=f"ytmp{do}")
                    nc.scalar.mul(ytmp[:], ps_y[:], 1.0 / (WS * WS3))
                    outt = P4.tile([128, T], F32, tag="outt", bufs=2,
                                   name=f"outt{do}")
                    nc.vector.tensor_add(outt[:], ytmp[:], hT[do][:])
                    nc.sync.dma_start(outT[do], outt[:])

    nc.compile()
    return nc


def prep_inputs(x, mask, wq, wk, wv, wo, w1, w2, w3, g_attn, g_ffn):
    """Build the 8 per-core input maps (host-side sharding + layout)."""
    bf = ml_dtypes.bfloat16
    f8 = ml_dtypes.float8_e4m3

    def q8(a, scale):
        return np.clip(a * scale, -240.0, 240.0).astype(f8)

    # wXc[p, cp, i, n] = ws * g[d] * wX[n, d],  d = (2cp+i)*128 + p
    def qkv_pack(w, ws):
        wt = (w * g_attn[None, :]).T.reshape(NCP, 2, 128, D)  # [cp, i, p, n]
        return np.ascontiguousarray(q8(wt.transpose(2, 0, 1, 3), ws))

    wqc = qkv_pack(wq, WS)
    wkc = qkv_pack(wk, WS)
    wvc = qkv_pack(wv, WSV)
    woTe = np.ascontiguousarray(q8(wo.T.reshape(NCH, 128, D), WS))

    # w1c[f, p, cp, (i, m)] = 64 * g_ffn[d] * w1[f*128+m, d]
    def ffn_pack(w, ws):
        wt = (w * g_ffn[None, :]).T.reshape(NCP, 2, 128, NFC, 128)
        return np.ascontiguousarray(
            q8(wt.transpose(3, 2, 0, 1, 4), ws).reshape(NFC, 128, NCP, 256))

    w1ce = ffn_pack(w1, WS)
    w3ce = ffn_pack(w3, WS3)
    # w2c[do, p, fp, (i, n)] = 64 * w2[do*128+n, (2fp+i)*128 + p]
    w2t = w2.T.reshape(NFP, 2, 128, NCH, 128)           # [fp, i, p, do, n]
    w2ce = np.ascontiguousarray(
        q8(w2t.transpose(3, 2, 0, 1, 4), WS).reshape(NCH, 128, NFP, 256))
    ones16 = np.ones((128, 128), bf)

    in_maps = []
    for core in range(8):
        b, qt = core // NQT, core % NQT
        # rotate tokens so the local 512-query slice is always quarter 0
        order = (np.arange(S) + qt * T) % S
        xb = x[b][order]                       # [S, D] rotated
        # xc[cp, p, i, t] = xb[t, (2cp+i)*128 + p]
        xce = np.ascontiguousarray(
            xb.T.reshape(NCP, 2, 128, S).transpose(0, 2, 1, 3)).astype(bf)
        xlocTe = np.ascontiguousarray(
            xb[0:T].T.reshape(NCH, 128, T)).astype(np.float32)
        # maskd[kt, p, i*T+q] = mask[b, qt*T+q, k], k = kt*128+p rotated keys
        msl = mask[b, qt * T:(qt + 1) * T][:, order]     # [T(q), S(k)] rotated
        maskde = np.ascontiguousarray(
            msl.T.reshape(NKT, 128, T)).astype(bf)       # [kt, p, q]
        in_maps.append({
            "xc": xce, "xlocT": xlocTe, "maskd": maskde,
            "wqc": wqc, "wkc": wkc, "wvc": wvc, "woT": woTe,
            "w1c": w1ce, "w3c": w3ce, "w2c": w2ce,
            "ones16": ones16,
        })
    return in_maps


_NC_CACHE = None


def get_nc():
    global _NC_CACHE
    if _NC_CACHE is None:
        _NC_CACHE = build_nc()
    return _NC_CACHE


def gather_output(results):
    out = np.empty((B, S, D), np.float32)
    for core in range(8):
        b, qt = core // NQT, core % NQT
        o = results[core]["outT"]              # [NCH, 128, T]
        out[b, qt * T:(qt + 1) * T, :] = o.reshape(D, T).T
    return out


def kernel(**inputs):
    from concourse.bass_utils import run_bass_kernel_spmd
    in_maps = prep_inputs(
        np.asarray(inputs["x"]), np.asarray(inputs["mask"]),
        np.asarray(inputs["wq"]), np.asarray(inputs["wk"]),
        np.asarray(inputs["wv"]), np.asarray(inputs["wo"]),
        np.asarray(inputs["w1"]), np.asarray(inputs["w2"]),
        np.asarray(inputs["w3"]),
        np.asarray(inputs["g_attn"]), np.asarray(inputs["g_ffn"]))
    nc = get_nc()
    res = run_bass_kernel_spmd(nc, in_maps, core_ids=list(range(8)))
    return gather_output(res.results)
